# revision 23
# baseline (speedup 1.0000x reference)
"""GNN message-passing (scatter-add) kernel for 8 Trainium2 NeuronCores.

Computes out = segment_sum(x[src], dst, num_segments=N) for
x [10000, 128] f32, edge_index [2, 320000] int64.

Strategy — dense count-matrix matmul (no gathers, no collectives):
  out[d] = sum_s A[s, d] * x[s]   with A[s, d] = #edges s->d.

  - Host computes A (np.bincount over (src, dst) pairs) and shards its
    columns: core c owns dst range [c*1264, (c+1)*1264). A entries are
    small ints, exact in fp8e4 (<=16); larger counts split into extra
    passes (never triggers for random graphs).
  - On device, out^T[f, d] = sum_k x_k^T-stationary @ A_k-moving.
    The dst range is split into tiles (512|512|240) processed
    TILE-MAJOR: all 79 source chunks for tile 0, then tile 1, then the
    cheap 240 tile. Each tile's PSUM drain (DVE copy + out DMA) hides
    under the next tile's matmuls, so only the tiny 240-tile drain is
    exposed at the end.
  - A loads ride the sync HWDGE queue alone (12.8MB, in exact PE
    consumption order); xt (2.6MB, 3 big slices) and the fp16 out
    stores ride the scalar HWDGE queue, so the A stream never stalls.
  - A PE warmup burst (dependency-free dummy matmuls) covers the DMA
    ramp and opens the HAM clock gate before real data arrives.
  - fp16 x keeps relative L2 error ~2e-4; A is exact, PSUM accumulates
    in f32; out is written fp16 (adds ~2e-4 error, halves out bytes).
  - Host transposes/concats the 8 cores' out^T tiles back to [10000, 128].

Per-core traffic: A ~12.8MB + x 2.6MB + out 0.32MB ~= 15.7MB.
"""

import sys

for _p in ("/opt/trn_rl_repo",):
    if _p not in sys.path:
        sys.path.append(_p)

import ml_dtypes
import numpy as np

import concourse.bacc as bacc
import concourse.mybir as mybir
import concourse.tile as tile
from concourse.bass_utils import run_bass_kernel_spmd

N_NODES = 10000
D_FEAT = 128
N_CORES = 8
P = 128
KCH = -(-N_NODES // P)  # 79 source chunks
NPAD = KCH * P  # 10112 (source rows padded; dst needs no padding)
DCORE = NPAD // N_CORES  # 1264 dst columns per core (16B-aligned A rows)
DTILES = [(0, 512), (512, 512), (1024, DCORE - 1024)]
GN = 8  # steady-state source-chunk load-group size


def _groups(sizes):
    out = []
    k0 = 0
    for g in sizes:
        out.append((k0, g))
        k0 += g
    assert k0 == KCH
    return out


# Tile 0 ramps its group sizes so the very first A load (1 chunk, 65KB)
# lands early and the PE starts cold matmuls ~5us sooner; later tiles use
# uniform groups (the DMA stream is far ahead by then).
KGROUPS_RAMP = _groups([1, 2, 4] + [8] * 9)
KGROUPS_FLAT = _groups([8] * 9 + [7])
# xt slice boundaries on the scalar queue, similarly ramped
XT_BOUNDS = [0, 2, 10, 30, KCH]
FP8 = ml_dtypes.float8_e4m3
FP8_MAX_INT = 16

# test/profiling hooks
TRACE = False
TRACE_CORES = None
LAST_RESULT = None


def _build_program(n_passes: int):
    nc = bacc.Bacc(
        "TRN2", target_bir_lowering=False, debug=False, num_devices=N_CORES
    )
    xt_d = nc.dram_tensor(
        "xt", [P, KCH * D_FEAT], mybir.dt.float16, kind="ExternalInput"
    )
    a_ds = [
        [
            nc.dram_tensor(
                f"a{ip}_{t}", [P, KCH * w], mybir.dt.float8e4, kind="ExternalInput"
            )
            for t, (off, w) in enumerate(DTILES)
        ]
        for ip in range(n_passes)
    ]
    o_d = nc.dram_tensor("o", [P, DCORE], mybir.dt.float16, kind="ExternalOutput")

    with tile.TileContext(nc) as tc:
        with (
            tc.tile_pool(name="xt", bufs=1) as xtp,
            tc.tile_pool(name="a", bufs=6) as ap_,
            tc.tile_pool(name="res", bufs=2) as resp,
            tc.tile_pool(name="ps", bufs=1, space="PSUM") as psp,
        ):
            xv = xt_d[:].rearrange("p (k f) -> p k f", k=KCH, f=D_FEAT)
            # xt: ramped slices on the scalar queue, issued first so they
            # overlap the A stream on the sync queue.
            kb = XT_BOUNDS
            xt_sb = xtp.tile(
                [P, KCH, D_FEAT], mybir.dt.float16, tag="xt", name="xt_sb"
            )
            for i in range(len(kb) - 1):
                nc.scalar.dma_start(
                    out=xt_sb[:, kb[i] : kb[i + 1], :],
                    in_=xv[:, kb[i] : kb[i + 1], :],
                )
            # No PE warmup: the PE sequencer comes up at ~7.5us, by which
            # time the first A groups (~3.7us) are already resident — real
            # matmuls at the HAM-cold rate open the clock gate just as fast
            # as dummy ones would, while doing useful work.
            pss = [
                psp.tile([P, w], mybir.dt.float32, tag=f"ps{t}", name=f"ps{t}")
                for t, (off, w) in enumerate(DTILES)
            ]

            def drain(t):
                off, w = DTILES[t]
                res = resp.tile(
                    [P, w], mybir.dt.float16, tag="res", name=f"res{t}"
                )
                nc.vector.tensor_copy(res[:], pss[t][:])
                nc.scalar.dma_start(out=o_d[:, off : off + w], in_=res[:])

            # tile-major phases: the wide tile first, then the cheap 240
            # tile (the wide tile's drain hides under the 240 matmuls)
            for t, (off, w) in enumerate(DTILES):
                for ip in range(n_passes):
                    av = a_ds[ip][t][:].rearrange(
                        "p (k w) -> p k w", k=KCH, w=w
                    )
                    kgroups = (
                        KGROUPS_RAMP if (t == 0 and ip == 0) else KGROUPS_FLAT
                    )
                    for gi, (k0, gn) in enumerate(kgroups):
                        a_sb = ap_.tile(
                            [P, gn, w],
                            mybir.dt.float8e4,
                            tag=f"a{t}_{gn}",
                            name=f"a{ip}_{t}_{gi}",
                        )
                        nc.sync.dma_start(
                            out=a_sb[:], in_=av[:, k0 : k0 + gn, :]
                        )
                        for kk in range(gn):
                            nc.tensor.matmul(
                                pss[t][:],
                                xt_sb[:, k0 + kk, :],
                                a_sb[:, kk, :],
                                start=(ip == 0 and k0 + kk == 0),
                                stop=(ip == n_passes - 1 and k0 + kk == KCH - 1),
                            )
                drain(t)

    nc.compile()
    return nc


def _prepare(x: np.ndarray, edge_index: np.ndarray):
    ei = np.asarray(edge_index)  # pull to host before any indexing
    src = ei[0].astype(np.int64)
    dst = ei[1].astype(np.int64)

    xf = np.asarray(x).astype(np.float32)
    xp = np.zeros((NPAD, D_FEAT), np.float16)
    xp[:N_NODES] = xf
    # xt[p, k, :] = x[k*128 + p, :]
    xt = np.ascontiguousarray(
        xp.reshape(KCH, P, D_FEAT).transpose(1, 0, 2).reshape(P, KCH * D_FEAT)
    )

    per_core_As = []
    n_passes = 1
    for c in range(N_CORES):
        sel = (dst >= c * DCORE) & (dst < (c + 1) * DCORE)
        idx = src[sel] * DCORE + (dst[sel] - c * DCORE)
        cnt = np.bincount(idx, minlength=NPAD * DCORE).reshape(NPAD, DCORE)
        passes = []
        while True:
            part = np.minimum(cnt, FP8_MAX_INT)
            # [P, KCH, DCORE] view, tile-major slices
            pk = part.astype(FP8).reshape(KCH, P, DCORE).transpose(1, 0, 2)
            passes.append(
                [
                    np.ascontiguousarray(pk[:, :, off : off + w]).reshape(P, KCH * w)
                    for (off, w) in DTILES
                ]
            )
            cnt = cnt - part
            if not cnt.any():
                break
        per_core_As.append(passes)
        n_passes = max(n_passes, len(passes))

    zeros = {}
    in_maps = []
    for c in range(N_CORES):
        m = {"xt": xt}
        for ip in range(n_passes):
            for t, (off, w) in enumerate(DTILES):
                if ip < len(per_core_As[c]):
                    m[f"a{ip}_{t}"] = per_core_As[c][ip][t]
                else:
                    if w not in zeros:
                        zeros[w] = np.zeros((P, KCH * w), FP8)
                    m[f"a{ip}_{t}"] = zeros[w]
        in_maps.append(m)
    return in_maps, n_passes


def kernel(x: np.ndarray, edge_index: np.ndarray) -> np.ndarray:
    global LAST_RESULT
    in_maps, n_passes = _prepare(x, edge_index)
    nc = _build_program(n_passes)
    res = run_bass_kernel_spmd(
        nc,
        in_maps,
        list(range(N_CORES)),
        trace=TRACE,
        trace_cores=TRACE_CORES if TRACE else None,
    )
    LAST_RESULT = res
    # o per core: [128 f, DCORE d] fp16 -> out[c*DCORE + d, f] f32
    out = np.concatenate(
        [np.asarray(r["o"]).astype(np.float32).T for r in res.results], axis=0
    )
    return np.ascontiguousarray(out[:N_NODES])


if __name__ == "__main__":
    rng = np.random.default_rng(0)
    x = rng.standard_normal((N_NODES, D_FEAT), dtype=np.float32)
    edge_index = rng.integers(0, N_NODES, size=(2, 320000)).astype(np.int64)
    out = kernel(x, edge_index)
    ref = np.zeros((N_NODES, D_FEAT), np.float32)
    np.add.at(ref, edge_index[1], x[edge_index[0]])
    rel = np.linalg.norm(out - ref) / np.linalg.norm(ref)
    print("rel L2 err:", rel)
